# revision 6
# baseline (speedup 1.0000x reference)
"""Trainium2 Bass kernel for nn_Discriminator (hetero GNN, 3 GraphConv layers
+ mean-pool head), distributed over 8 NeuronCores.

Strategy (dst-sharding):
  - Nodes are partitioned into 8 contiguous shards of 12800; core c owns the
    edges whose dst lands in its shard (for every relation).
  - Per-edge normalization rsqrt(deg_out[src]) * rsqrt(deg_in[dst]) is folded
    into a single per-edge scalar s_e on the host, so on-device aggregation is
    just sum_e s_e * x[src_e].
  - Aggregation: edges are dst-sorted and chunked 128-at-a-time; each chunk is
    gathered with one indirect DMA, scaled by s_e (DVE), turned into a one-hot
    matrix A[e, dst_local] (DVE is_equal vs iota), and scatter-accumulated on
    the TensorEngine: psum[feat, dst] += lhsT(msgs).T @ A.
    The transposed [feat, dst] result is exactly the stationary operand for
    the following dense matmul, so no transposes are ever needed.
  - Layer 1 keeps the 3 relations separate through aggregation (per-relation
    W1), packed into one PSUM tile [64, 3*128]. Layers 2/3 pre-apply W per
    relation (y_r = x @ W2_r) and merge all relations into one aggregation
    (gather indices point into the all-gathered concat of y_r shards).
  - Between layers: AllGather of the per-core y_r / z_r shards (the only
    communication), plus a tiny AllGather for the pooled head.
  - Each layer is a hardware For_i loop over the 100 dst blocks with a
    uniform per-block chunk schedule (max over blocks/cores, s_e=0 padding):
    the loop back-edge resets semaphores (required: >16k DMAs would overflow
    the 16-bit semaphore wait fields in a fully unrolled program).
"""

import os
import numpy as np

from concourse import bass, bacc, tile, mybir
from concourse import bass_utils

# problem constants (hardcoded per contract)
N = 102400
R = 3
E = 819200
NCORES = 8
NSH = N // NCORES          # 12800 nodes per shard
P = 128
NBLK = NSH // P            # 100 dst blocks per core
F0, F1, F2, F3 = 64, 256, 128, 64
NQ = N // 64               # 1600 head rows
NQSH = NSH // 64           # 200 head rows per core

FP = mybir.dt.float32
# dtype for the scatter matmul operands (msgs + one-hot): bf16 halves
# DVE/SBUF cost at ~0.4% msg rounding
MM_DT = mybir.dt.bfloat16 if os.environ.get("K_BF16_MM") else mybir.dt.float32
# dtype of the allgathered Y/Z gather sources (halves collective volume)
AG_DT = mybir.dt.bfloat16 if os.environ.get("K_BF16_AG") else mybir.dt.float32
UNROLL = int(os.environ.get("K_UNROLL", "4"))


# ---------------------------------------------------------------- host prep

def _pack_uniform(dst_local_sorted, vals, nblk, ch):
    """Pack dst-sorted edges into [128, nblk*ch] slot arrays (s=0 padding)."""
    blk = dst_local_sorted // P
    counts = np.bincount(blk, minlength=nblk)
    out = {k: np.zeros((P, nblk * ch), v.dtype) for k, v in vals.items()}
    out["dl"] = np.zeros((P, nblk * ch), np.float32)
    e0 = np.concatenate([[0], np.cumsum(counts)])[:-1]
    for b in range(nblk):
        cnt = int(counts[b])
        sl = slice(e0[b], e0[b] + cnt)
        buf = np.zeros(ch * P, np.float32)
        buf[:cnt] = (dst_local_sorted[sl] % P).astype(np.float32)
        out["dl"][:, b * ch:(b + 1) * ch] = buf.reshape(ch, P).T
        for k, v in vals.items():
            bufv = np.zeros(ch * P, v.dtype)
            bufv[:cnt] = v[sl]
            out[k][:, b * ch:(b + 1) * ch] = bufv.reshape(ch, P).T
    return out


def preprocess(h, src, dst, W1, b1, W2, b2, W3, b3, Wd, bd):
    """Build per-core metadata + packed weights. Returns (sched, in_maps)."""
    src = np.asarray(src)
    dst = np.asarray(dst)
    s_e = np.empty((R, E), np.float32)
    for r in range(R):
        deg_out = np.maximum(np.bincount(src[r], minlength=N), 1).astype(np.float32)
        deg_in = np.maximum(np.bincount(dst[r], minlength=N), 1).astype(np.float32)
        s_e[r] = (1.0 / np.sqrt(deg_out[src[r]])) * (1.0 / np.sqrt(deg_in[dst[r]]))

    core_of = dst // NSH
    dloc = dst % NSH

    per_core_l1 = []
    per_core_l23 = []
    cnts1 = np.zeros((NCORES, R, NBLK), np.int64)
    cnts2 = np.zeros((NCORES, NBLK), np.int64)
    for c in range(NCORES):
        l1 = []
        m_idx, m_s, m_dl = [], [], []
        for r in range(R):
            sel = np.nonzero(core_of[r] == c)[0]
            dl = dloc[r][sel]
            order = np.argsort(dl, kind="stable")
            dl_s = dl[order]
            idx_s = src[r][sel][order].astype(np.int32)
            s_s = s_e[r][sel][order]
            l1.append((idx_s, s_s, dl_s))
            cnts1[c, r] = np.bincount(dl_s // P, minlength=NBLK)
            idx2 = ((idx_s // NSH) * (R * NSH) + r * NSH + (idx_s % NSH)).astype(np.int32)
            m_idx.append(idx2)
            m_s.append(s_s)
            m_dl.append(dl_s)
        m_idx = np.concatenate(m_idx)
        m_s = np.concatenate(m_s)
        m_dl = np.concatenate(m_dl)
        order = np.argsort(m_dl, kind="stable")
        per_core_l1.append(l1)
        per_core_l23.append((m_idx[order], m_s[order], m_dl[order]))
        cnts2[c] = np.bincount(m_dl[order] // P, minlength=NBLK)

    # uniform per-block chunk counts (shared across cores AND blocks)
    ch1 = tuple(int(x) for x in
                np.maximum((cnts1 + P - 1) // P, 1).max(axis=(0, 2)))   # per r
    ch2 = int(np.maximum((cnts2 + P - 1) // P, 1).max())

    in_maps = []
    iota = np.tile(np.arange(P, dtype=np.float32), (P, 1))
    W1p = np.transpose(W1, (1, 0, 2)).reshape(64, R * 256).astype(np.float32)
    W2p = np.transpose(W2, (1, 0, 2))                 # [256, R, 128]
    W2p = np.concatenate([W2p[:128], W2p[128:]], axis=1)  # [128, 2R, 128]
    W2p = W2p.reshape(128, 2 * R * 128).astype(np.float32)
    W3p = np.transpose(W3, (1, 0, 2)).reshape(128, R * 64).astype(np.float32)
    b1p = b1.sum(axis=0).astype(np.float32).reshape(2, 128).T.copy()
    b2p = b2.sum(axis=0).astype(np.float32).reshape(128, 1)
    b3p = np.zeros((128, 1), np.float32)
    b3p[:64, 0] = b3.sum(axis=0)
    onesp = np.zeros((128, 1), np.float32)
    onesp[:64, 0] = 1.0
    u = (Wd[:, 0].astype(np.float32) / 64.0).reshape(1, 64)
    bdp = np.asarray(bd, np.float32).reshape(1, 1)

    CB1 = sum(ch1)
    for c in range(NCORES):
        packs = []
        for r in range(R):
            idx_s, s_s, dl_s = per_core_l1[c][r]
            packs.append(_pack_uniform(dl_s, {"idx": idx_s, "s": s_s},
                                       NBLK, ch1[r]))
        idx1 = np.zeros((P, NBLK * CB1), np.int32)
        s1 = np.zeros((P, NBLK * CB1), np.float32)
        dl1 = np.zeros((P, NBLK * CB1), np.float32)
        for b in range(NBLK):
            o = b * CB1
            for r in range(R):
                sl = slice(b * ch1[r], (b + 1) * ch1[r])
                idx1[:, o:o + ch1[r]] = packs[r]["idx"][:, sl]
                s1[:, o:o + ch1[r]] = packs[r]["s"][:, sl]
                dl1[:, o:o + ch1[r]] = packs[r]["dl"][:, sl]
                o += ch1[r]

        m_idx, m_s, m_dl = per_core_l23[c]
        p2 = _pack_uniform(m_dl, {"idx": m_idx, "s": m_s}, NBLK, ch2)
        in_maps.append({
            "h": np.asarray(h, np.float32),
            "idx1": idx1, "s1": s1, "dl1": dl1,
            "idx2": p2["idx"].astype(np.int32),
            "s2": p2["s"].astype(np.float32), "dl2": p2["dl"],
            "iota": iota, "W1p": W1p, "W2p": W2p, "W3p": W3p,
            "b1p": b1p, "b2p": b2p, "b3p": b3p, "onesp": onesp,
            "u": u, "bdp": bdp,
        })
    return (ch1, ch2), in_maps


# ---------------------------------------------------------------- kernel IR

def build_kernel(ch1, ch2):
    CB1 = sum(ch1)
    nc = bacc.Bacc("TRN2", target_bir_lowering=False, debug=False,
                   enable_asserts=False, num_devices=NCORES)

    t_h = nc.dram_tensor("h", [N, F0], FP, kind="ExternalInput")
    t_idx1 = nc.dram_tensor("idx1", [P, NBLK * CB1], mybir.dt.int32, kind="ExternalInput")
    t_s1 = nc.dram_tensor("s1", [P, NBLK * CB1], FP, kind="ExternalInput")
    t_dl1 = nc.dram_tensor("dl1", [P, NBLK * CB1], FP, kind="ExternalInput")
    t_idx2 = nc.dram_tensor("idx2", [P, NBLK * ch2], mybir.dt.int32, kind="ExternalInput")
    t_s2 = nc.dram_tensor("s2", [P, NBLK * ch2], FP, kind="ExternalInput")
    t_dl2 = nc.dram_tensor("dl2", [P, NBLK * ch2], FP, kind="ExternalInput")
    t_iota = nc.dram_tensor("iota", [P, P], FP, kind="ExternalInput")
    t_W1p = nc.dram_tensor("W1p", [64, R * 256], FP, kind="ExternalInput")
    t_W2p = nc.dram_tensor("W2p", [128, 2 * R * 128], FP, kind="ExternalInput")
    t_W3p = nc.dram_tensor("W3p", [128, R * 64], FP, kind="ExternalInput")
    t_b1p = nc.dram_tensor("b1p", [P, 2], FP, kind="ExternalInput")
    t_b2p = nc.dram_tensor("b2p", [P, 1], FP, kind="ExternalInput")
    t_b3p = nc.dram_tensor("b3p", [P, 1], FP, kind="ExternalInput")
    t_ones = nc.dram_tensor("onesp", [P, 1], FP, kind="ExternalInput")
    t_u = nc.dram_tensor("u", [1, 64], FP, kind="ExternalInput")
    t_bdp = nc.dram_tensor("bdp", [1, 1], FP, kind="ExternalInput")
    t_out = nc.dram_tensor("out", [NQ, 1], FP, kind="ExternalOutput")

    with tile.TileContext(nc) as tc:
        with (
            tc.tile_pool(name="const", bufs=1) as const,
            tc.tile_pool(name="meta", bufs=3) as meta,
            tc.tile_pool(name="stage", bufs=2) as stage,
            tc.tile_pool(name="sbw", bufs=3) as sbw,
            tc.tile_pool(name="ps_sc", bufs=2, space="PSUM") as ps_sc,
            tc.tile_pool(name="ps_d", bufs=2, space="PSUM") as ps_d,
            tc.tile_pool(name="dram", bufs=1, space="DRAM") as dram,
        ):
            iota_t = const.tile([P, P], FP)
            nc.sync.dma_start(iota_t[:], t_iota.ap())
            W1t = const.tile([64, R * 256], FP)
            nc.sync.dma_start(W1t[:], t_W1p.ap())
            W2t = const.tile([128, 2 * R * 128], FP)
            nc.sync.dma_start(W2t[:], t_W2p.ap())
            W3t = const.tile([128, R * 64], FP)
            nc.sync.dma_start(W3t[:], t_W3p.ap())
            b1t = const.tile([P, 2], FP)
            nc.sync.dma_start(b1t[:], t_b1p.ap())
            b2t = const.tile([P, 1], FP)
            nc.sync.dma_start(b2t[:], t_b2p.ap())
            b3t = const.tile([P, 1], FP)
            nc.sync.dma_start(b3t[:], t_b3p.ap())
            ones_t = const.tile([P, 1], FP)
            nc.sync.dma_start(ones_t[:], t_ones.ap())
            u_t = const.tile([1, 64], FP)
            nc.sync.dma_start(u_t[:], t_u.ap())
            bd_t = const.tile([1, 1], FP)
            nc.sync.dma_start(bd_t[:], t_bdp.ap())
            q_sb = const.tile([1, NQSH], FP, tag="q_sb")

            y_loc = dram.tile([R * NSH, F2], AG_DT)
            Y_all = dram.tile([NCORES * R * NSH, F2], AG_DT, addr_space="Shared")
            z_loc = dram.tile([R * NSH, F3], AG_DT)
            Z_all = dram.tile([NCORES * R * NSH, F3], AG_DT, addr_space="Shared")
            vq_loc = dram.tile([1, NQSH], FP)
            V_all = dram.tile([1, NQ], FP, addr_space="Shared")

            def gather_scale_onehot(t_idx, t_s, t_dl, src_ap, i, ncols, fdim,
                                    tagp, src_dt=FP):
                mi = meta.tile([P, ncols], mybir.dt.int32, tag="mi")
                nc.sync.dma_start(mi[:], t_idx.ap()[:, bass.ts(i, ncols)])
                ms = meta.tile([P, ncols], FP, tag="ms")
                nc.sync.dma_start(ms[:], t_s.ap()[:, bass.ts(i, ncols)])
                md = meta.tile([P, ncols], FP, tag="md")
                nc.sync.dma_start(md[:], t_dl.ap()[:, bass.ts(i, ncols)])

                msgs = stage.tile([P, ncols * fdim], MM_DT, tag="msgs")
                graw = stage.tile([P, ncols * fdim], src_dt, tag="graw")
                for j in range(ncols):
                    nc.gpsimd.indirect_dma_start(
                        out=graw[:, j * fdim:(j + 1) * fdim],
                        out_offset=None, in_=src_ap,
                        in_offset=bass.IndirectOffsetOnAxis(
                            ap=mi[:, j:j + 1], axis=0))
                nc.vector.tensor_tensor(
                    out=msgs[:].rearrange("p (k f) -> p k f", k=ncols),
                    in0=graw[:].rearrange("p (k f) -> p k f", k=ncols),
                    in1=ms[:].to_broadcast([P, ncols, fdim]),
                    op=mybir.AluOpType.mult)
                A = stage.tile([P, ncols * P], MM_DT, tag="A")
                nc.vector.tensor_tensor(
                    out=A[:].rearrange("p (k d) -> p k d", k=ncols),
                    in0=md[:].to_broadcast([P, ncols, P]),
                    in1=iota_t[:].rearrange("p (o d) -> p o d", o=1)
                        .to_broadcast([P, ncols, P]),
                    op=mybir.AluOpType.is_equal)
                return msgs, A

            # ================= Layer 1 =================
            def l1_body(i):
                msgs, A = gather_scale_onehot(t_idx1, t_s1, t_dl1, t_h.ap(),
                                              i, CB1, F0, "1")
                ps = ps_sc.tile([64, R * P], FP, space="PSUM", tag="sc")
                col = 0
                for r in range(R):
                    for j in range(ch1[r]):
                        nc.tensor.matmul(
                            out=ps[:, r * P:(r + 1) * P],
                            lhsT=msgs[:, col * F0:(col + 1) * F0],
                            rhs=A[:, col * P:(col + 1) * P],
                            start=(j == 0), stop=(j == ch1[r] - 1))
                        col += 1
                mT = sbw.tile([64, R * P], FP, tag="mT1")
                nc.vector.tensor_copy(out=mT[:], in_=ps[:])
                x1ps = ps_d.tile([128, 256], FP, space="PSUM", tag="d")
                for half in (0, 1):
                    for r in range(R):
                        nc.tensor.matmul(
                            out=x1ps[:, half * 128:(half + 1) * 128],
                            lhsT=W1t[:, r * 256 + half * 128:
                                     r * 256 + (half + 1) * 128],
                            rhs=mT[:, r * P:(r + 1) * P],
                            start=(r == 0), stop=(r == R - 1))
                x1T = sbw.tile([128, 256], FP, tag="x1T_sb")
                for half in (0, 1):
                    nc.vector.tensor_scalar(
                        out=x1T[:, half * 128:(half + 1) * 128],
                        in0=x1ps[:, half * 128:(half + 1) * 128],
                        scalar1=b1t[:, half:half + 1], scalar2=None,
                        op0=mybir.AluOpType.add)
                for r in range(R):
                    yps = ps_d.tile([128, 128], FP, space="PSUM", tag="d")
                    for half in (0, 1):
                        nc.tensor.matmul(
                            out=yps[:],
                            lhsT=x1T[:, half * 128:(half + 1) * 128],
                            rhs=W2t[:, (half * R + r) * 128:
                                    (half * R + r + 1) * 128],
                            start=(half == 0), stop=(half == 1))
                    ysb = sbw.tile([128, 128], AG_DT, tag="ysb")
                    nc.vector.tensor_copy(out=ysb[:], in_=yps[:])
                    nc.sync.dma_start(
                        y_loc[bass.ds(i * P + r * NSH, P), :], ysb[:])

            tc.For_i_unrolled(0, NBLK, 1, l1_body, max_unroll=UNROLL)

            nc.gpsimd.collective_compute(
                "AllGather", mybir.AluOpType.bypass,
                replica_groups=[list(range(NCORES))],
                ins=[y_loc[:]], outs=[Y_all[:]])

            # ================= Layer 2 =================
            def l2_body(i):
                msgs, A = gather_scale_onehot(t_idx2, t_s2, t_dl2, Y_all[:],
                                              i, ch2, F2, "2", src_dt=AG_DT)
                ps = ps_sc.tile([128, P], FP, space="PSUM", tag="sc")
                for j in range(ch2):
                    nc.tensor.matmul(
                        out=ps[:],
                        lhsT=msgs[:, j * F2:(j + 1) * F2],
                        rhs=A[:, j * P:(j + 1) * P],
                        start=(j == 0), stop=(j == ch2 - 1))
                x2T = sbw.tile([128, P], FP, tag="x2T")
                nc.vector.tensor_scalar(
                    out=x2T[:], in0=ps[:], scalar1=b2t[:, 0:1], scalar2=None,
                    op0=mybir.AluOpType.add)
                zps = ps_d.tile([128, R * 64], FP, space="PSUM", tag="d")
                for r in range(R):
                    nc.tensor.matmul(
                        out=zps[:, r * 64:(r + 1) * 64],
                        lhsT=x2T[:],
                        rhs=W3t[:, r * 64:(r + 1) * 64],
                        start=True, stop=True)
                zsb = sbw.tile([128, R * 64], AG_DT, tag="zsb")
                nc.vector.tensor_copy(out=zsb[:], in_=zps[:])
                for r in range(R):
                    nc.sync.dma_start(
                        z_loc[bass.ds(i * P + r * NSH, P), :],
                        zsb[:, r * 64:(r + 1) * 64])

            tc.For_i_unrolled(0, NBLK, 1, l2_body, max_unroll=UNROLL)

            nc.gpsimd.collective_compute(
                "AllGather", mybir.AluOpType.bypass,
                replica_groups=[list(range(NCORES))],
                ins=[z_loc[:]], outs=[Z_all[:]])

            # ================= Layer 3 + head =================
            def l3_body(i):
                msgs, A = gather_scale_onehot(t_idx2, t_s2, t_dl2, Z_all[:],
                                              i, ch2, F3, "3", src_dt=AG_DT)
                ps = ps_sc.tile([64, P], FP, space="PSUM", tag="sc")
                for j in range(ch2):
                    nc.tensor.matmul(
                        out=ps[:],
                        lhsT=msgs[:, j * F3:(j + 1) * F3],
                        rhs=A[:, j * P:(j + 1) * P],
                        start=(j == 0), stop=(j == ch2 - 1))
                x3T = sbw.tile([64, P], FP, tag="x3T")
                nc.vector.tensor_scalar(
                    out=x3T[:], in0=ps[:], scalar1=b3t[:64, 0:1], scalar2=None,
                    op0=mybir.AluOpType.add)
                vps = ps_d.tile([1, P], FP, space="PSUM", tag="d")
                nc.tensor.matmul(out=vps[:], lhsT=ones_t[:64, 0:1],
                                 rhs=x3T[:], start=True, stop=True)
                wv = sbw.tile([1, P], FP, tag="wv")
                nc.vector.tensor_tensor(
                    out=wv[:].rearrange("o (x f) -> o x f", f=64),
                    in0=vps[:].rearrange("o (x f) -> o x f", f=64),
                    in1=u_t[:].rearrange("o (x f) -> o x f", x=1)
                        .to_broadcast([1, P // 64, 64]),
                    op=mybir.AluOpType.mult)
                nc.vector.reduce_sum(
                    out=q_sb[0:1, bass.ts(i, P // 64)]
                        .rearrange("o (q j) -> o q j", j=1),
                    in_=wv[:].rearrange("o (q j) -> o q j", j=64),
                    axis=mybir.AxisListType.X)

            tc.For_i_unrolled(0, NBLK, 1, l3_body, max_unroll=UNROLL)

            nc.sync.dma_start(vq_loc[:], q_sb[:])
            nc.gpsimd.collective_compute(
                "AllGather", mybir.AluOpType.bypass,
                replica_groups=[list(range(NCORES))],
                ins=[vq_loc[:]], outs=[V_all[:]])
            o_sb = sbw.tile([1, NQ], FP, tag="o_sb")
            nc.sync.dma_start(o_sb[:], V_all[:])
            o2_sb = sbw.tile([1, NQ], FP, tag="o2_sb")
            nc.vector.tensor_scalar(out=o2_sb[:], in0=o_sb[:],
                                    scalar1=bd_t[0:1, 0:1], scalar2=None,
                                    op0=mybir.AluOpType.add)
            nc.sync.dma_start(t_out.ap().rearrange("q one -> one q"),
                              o2_sb[:])

    nc.compile()
    return nc


# ---------------------------------------------------------------- entry

_CACHE = {}


def kernel(h, src, dst, W1, b1, W2, b2, W3, b3, Wd, bd):
    (ch1, ch2), in_maps = preprocess(h, src, dst, W1, b1, W2, b2,
                                     W3, b3, Wd, bd)
    key = (ch1, ch2)
    if key not in _CACHE:
        _CACHE[key] = build_kernel(ch1, ch2)
    nc = _CACHE[key]
    res = bass_utils.run_bass_kernel_spmd(nc, in_maps,
                                          core_ids=list(range(NCORES)))
    return res.results[0]["out"].astype(np.float32)
